# revision 61
# baseline (speedup 1.0000x reference)
"""Trainium2 Bass kernel for 16-head causal MHA (B=2, T=2048, C=1024, H=16, D=64).

Sharding: 8 cores = 2 batch groups x 4 head groups (4 heads each).
All matmuls run in bf16 (inputs pre-cast on host; fp32 PSUM accumulate).

v3: single fused pipeline keeping the PE dense and the ACT-bound exp stream
overlapped:
  - projection matmul groups for chunk n+1 and out-projection tiles for
    chunk n-1 drain as PE "filler" inside chunk n's attention t-loops
  - causal mask applied on the PE (identity-matmul accumulate of a
    triangular bf16 constant into the score PSUM)
  - AV output PSUM freed immediately via an SBUF bounce copy; softmax
    normalization (reciprocal_approx_fast + gpsimd partition_broadcast +
    DVE multiply) runs decoupled, with the multiplies deferred into the
    next loop so no engine head-of-line blocks on the broadcast
  - PE warm-up matmuls flip the HAM clock gate to 8/8 during the initial
    DMA wait
Host sums the 4 head-group partials per batch and adds bo.
"""

import sys

sys.path.insert(0, "/opt/trn_rl_repo")

import numpy as np
import ml_dtypes

import concourse.bass as bass
from concourse import bacc
import concourse.mybir as mybir
from concourse.tile import TileContext
from concourse.bass_utils import run_bass_kernel_spmd
from concourse.masks import make_identity

F32 = mybir.dt.float32
BF16 = mybir.dt.bfloat16
F8 = mybir.dt.float8e4
DR = mybir.MatmulPerfMode.DoubleRow
EXP = mybir.ActivationFunctionType.Exp

B, T, C, H, D = 2, 2048, 1024, 16, 64
NHPC = 4          # heads per core
DH = NHPC * D     # 256 head dims per core
P = 128           # partitions
CH = 512          # token chunk (matmul moving dim)
NCHUNK = T // CH  # 4
NTT = T // P      # 16 token tiles
NCT = C // P      # 8 contraction tiles over C
NEG = -30000.0    # masked-score fill; exp() flushes to 0


import os

_DISABLE = set(os.environ.get("KERNEL_DISABLE", "").split(","))


def build_nc():
    no_warmup = "warmup" in _DISABLE
    no_defer = "defer" in _DISABLE
    no_interleave = "interleave" in _DISABLE
    no_pemask = "pemask" in _DISABLE
    nc = bacc.Bacc()
    xT_d = nc.declare_dram_parameter("xT", [C, T], BF16, isOutput=False)
    wqkv_d = nc.declare_dram_parameter("Wqkv", [C, 3 * DH], BF16, isOutput=False)
    wot_d = nc.declare_dram_parameter("WoT", [DH, C], BF16, isOutput=False)
    y_d = nc.declare_dram_parameter("Y", [T, C], F32, isOutput=True)

    # batched-DMA views: one descriptor chain covers a whole chunk / weight
    xTr = xT_d[:, :].rearrange("(c p) t -> p c t", p=P)      # [128, 8, 2048]
    wqkvr = wqkv_d[:, :].rearrange("(c p) d -> p c d", p=P)  # [128, 8, 768]
    wotr = wot_d[:, :].rearrange("(k p) e -> p k e", p=P)    # [128, 2, 1024]
    y = y_d[:, :]

    with TileContext(nc) as tc:
        with (
            tc.tile_pool(name="const", bufs=1) as const,
            tc.tile_pool(name="persist", bufs=1) as persist,
            tc.tile_pool(name="xt", bufs=4) as xt_pool,
            tc.tile_pool(name="pt", bufs=4) as pt_pool,
            tc.tile_pool(name="small", bufs=4) as small_pool,
            tc.tile_pool(name="ysb", bufs=4) as y_pool,
            tc.tile_pool(name="psproj", bufs=2, space="PSUM") as ps_proj,
            tc.tile_pool(name="psst", bufs=2, space="PSUM") as ps_st,
            tc.tile_pool(name="psot", bufs=2, space="PSUM") as ps_ot,
        ):
            # ---- persistent weight tiles + batched input DMAs (issued
            # before the const init so data streams during setup) ----
            wq_all = persist.tile([P, NCT, 3 * DH], BF16, name="wqall")
            wot_all = persist.tile([P, 2, C], BF16, name="wotall")
            xts = {}  # chunk -> [128, 8, 512] tile

            def emit_x_dma(n):
                xtile = xt_pool.tile([P, NCT, CH], BF16, tag="xt",
                                     name=f"xt{n}")
                nc.sync.dma_start(xtile[:],
                                  xTr[:, :, n * CH : (n + 1) * CH])
                xts[n] = xtile

            emit_x_dma(0)
            # split so Q/K projections can start before the V block lands
            nc.sync.dma_start(wq_all[:, :, 0 : 2 * DH],
                              wqkvr[:, :, 0 : 2 * DH])
            nc.sync.dma_start(wq_all[:, :, 2 * DH : 3 * DH],
                              wqkvr[:, :, 2 * DH : 3 * DH])
            nc.sync.dma_start(wot_all[:], wotr[:, :, :])
            emit_x_dma(1)

            # ---- constants ----
            id_f32 = const.tile([P, P], F32, name="idf")
            make_identity(nc, id_f32[:])
            id128 = const.tile([P, P], BF16, name="id128")
            # strictly-upper-triangular NEG (transposed causal mask):
            # maskT[c, i] = NEG if i > c else 0
            maskT_f32 = const.tile([P, P], F32, name="mtf")
            nc.gpsimd.memset(maskT_f32[:], 0.0)
            nc.gpsimd.affine_select(
                out=maskT_f32[:],
                in_=maskT_f32[:],
                compare_op=mybir.AluOpType.is_ge,
                fill=NEG,
                base=0,
                pattern=[[-1, P]],
                channel_multiplier=1,
            )
            maskT = const.tile([P, P], BF16, name="maskT")
            with nc.allow_low_precision("bf16 consts"):
                nc.vector.tensor_copy(id128[:], id_f32[:])
                nc.vector.tensor_copy(maskT[:], maskT_f32[:])
            ones_col = const.tile([1, D], F32, name="ones_col")
            nc.gpsimd.memset(ones_col[:], 1.0)
            if no_pemask:
                # DVE-side causal mask (baseline style):
                # mask128[r, (hh, j)] = 0 if r <= j else NEG
                mask128 = const.tile([P, 2, P], F32, name="mask128")
                nc.gpsimd.memset(mask128[:], 0.0)
                nc.gpsimd.affine_select(
                    out=mask128[:],
                    in_=mask128[:],
                    compare_op=mybir.AluOpType.is_ge,
                    fill=NEG,
                    base=0,
                    pattern=[[0, 2], [1, P]],
                    channel_multiplier=-1,
                )

            # ---- persistent tensors ----
            # Q^T/K^T [dims, tokens]; pair p holds heads (2p, 2p+1)
            qt_t = [persist.tile([P, T], BF16, name=f"qt{p}") for p in range(2)]
            kt_t = [persist.tile([P, T], BF16, name=f"kt{p}") for p in range(2)]
            # V augmented with a ones column per head: [tokens, 4, 65]
            vaug_t = [persist.tile([P, NHPC, D + 1], BF16, name=f"vaug{t}")
                      for t in range(NTT)]
            for t in range(NTT):
                nc.gpsimd.memset(vaug_t[t][:, :, D : D + 1], 1.0)
            # normalized O^T [dims, tokens]
            ot_t = [persist.tile([P, T], BF16, name=f"ot{p}") for p in range(2)]

            # ---- PE warm-up (runs during the initial DMA wait) ----
            if not no_warmup:
                warm = ps_proj.tile([P, CH], F32, tag="pp", name="warm")
                for _ in range(72):
                    nc.tensor.matmul(warm[:, 0:P], lhsT=id128[:], rhs=maskT[:],
                                     start=True, stop=True)

            # ---- stage-1 projection groups (PE filler units) ----
            def emit_proj_qk(n, m):
                csl = slice(n * CH, (n + 1) * CH)
                ps = ps_proj.tile([P, CH], F32, tag="pp", name=f"ps{n}_{m}")
                for c in range(NCT):
                    nc.tensor.matmul(
                        ps[:],
                        lhsT=wq_all[:, c, m * P : (m + 1) * P],
                        rhs=xts[n][:, c, :],
                        start=(c == 0),
                        stop=(c == NCT - 1),
                    )
                with nc.allow_low_precision("bf16 store"):
                    if m < 2:
                        nc.vector.tensor_copy(qt_t[m][:, csl], ps[:])
                    else:
                        nc.vector.tensor_copy(kt_t[m - 2][:, csl], ps[:])

            def emit_proj_v(n, j):
                vp = ps_proj.tile([P, CH], F32, tag="pp", name=f"vp{n}_{j}")
                for c in range(NCT):
                    nc.tensor.matmul(
                        vp[:, 0:DH],
                        lhsT=xts[n][:, c, j * P : (j + 1) * P],
                        rhs=wq_all[:, c, 2 * DH : 3 * DH],
                        start=(c == 0),
                        stop=(c == NCT - 1),
                    )
                va = vaug_t[4 * n + j]
                with nc.allow_low_precision("bf16 store"):
                    nc.vector.tensor_copy(
                        va[:, :, 0:D],
                        vp[:, 0:DH].rearrange("p (h d) -> p h d", h=NHPC))

            def proj_chunk_units(n):
                units = [(lambda m=m, n=n: emit_proj_qk(n, m)) for m in range(4)]
                units += [(lambda j=j, n=n: emit_proj_v(n, j)) for j in range(4)]
                return units

            # chunk 0 runs before the attention loops start
            for u in proj_chunk_units(0):
                u()

            # ---- out-projection tile (PE filler unit) ----
            ysb_tiles = {}

            def emit_outproj_half(tt, nn):
                tsl = slice(tt * P, (tt + 1) * P)
                if nn == 0:
                    ysb_tiles[tt] = y_pool.tile([P, 2, CH], F32, tag="ysb",
                                                name=f"ysb{tt}")
                ysb = ysb_tiles[tt]
                nsl = slice(nn * CH, (nn + 1) * CH)
                yp = ps_proj.tile([P, CH], F32, tag="pp", name=f"y{tt}_{nn}")
                for k in range(2):
                    nc.tensor.matmul(
                        yp[:],
                        lhsT=ot_t[k][:, tsl],
                        rhs=wot_all[:, k, nsl],
                        start=(k == 0),
                        stop=(k == 1),
                    )
                nc.vector.tensor_copy(ysb[:, nn, :], yp[:])
                if nn == 1:
                    nc.sync.dma_start(y[tsl, :], ysb[:])
                    del ysb_tiles[tt]

            def emit_outproj_tile(tt):
                emit_outproj_half(tt, 0)
                emit_outproj_half(tt, 1)

            # ---- fused attention with filler drain ----
            filler = []          # list of (kind, emit_fn)
            pending_mults = []   # deferred normalization multiplies

            def drain(kinds=("proj", "out"), limit=1):
                done = 0
                i = 0
                while i < len(filler) and done < limit:
                    kind, fn = filler[i]
                    if kind in kinds:
                        filler.pop(i)
                        fn()
                        done += 1
                    else:
                        i += 1

            for cq in range(NCHUNK):
                if cq + 1 <= NCHUNK - 1:
                    units = proj_chunk_units(cq + 1)
                    if no_interleave:
                        for u in units:
                            u()
                    else:
                        filler.extend(("proj", u) for u in units)
                if cq + 2 <= NCHUNK - 1:
                    emit_x_dma(cq + 2)
                qsl = slice(cq * CH, (cq + 1) * CH)
                nts = 4 * cq + 4

                def emit_st(t, cq=cq, p=None):
                    st = ps_st.tile([P, 2, CH], F32, tag="st",
                                    name=f"st{cq}_{p}_{t}")
                    tsl = slice(t * P, (t + 1) * P)
                    js = max(0, (t - 4 * cq) * P)
                    qs = slice(cq * CH + js, (cq + 1) * CH)
                    diag = t >= 4 * cq
                    for hh in range(2):
                        nc.tensor.matmul(
                            st[:, hh, js:],
                            lhsT=kt_t[p][hh * D : (hh + 1) * D, tsl],
                            rhs=qt_t[p][hh * D : (hh + 1) * D, qs],
                            start=True,
                            stop=True if no_pemask else not diag,
                        )
                    if diag:
                        if no_pemask:
                            nc.vector.tensor_add(
                                st[:, :, js : js + P],
                                st[:, :, js : js + P],
                                mask128[:],
                            )
                        else:
                            # causal mask on the PE: st[:,hh,js:js+P] += maskT.T
                            for hh in range(2):
                                nc.tensor.matmul(
                                    st[:, hh, js : js + P],
                                    lhsT=maskT[:],
                                    rhs=id128[:],
                                    start=False,
                                    stop=True,
                                )
                    return st, js

                for p in range(2):
                    ots = [
                        ps_ot.tile([D + 1, CH], F32, tag="ot",
                                   name=f"ot{cq}_{p}_{hh}")
                        for hh in range(2)
                    ]
                    sts = {0: emit_st(0, p=p)}
                    for t in range(nts):
                        if t == 2 and pending_mults:
                            for fn in pending_mults:
                                fn()
                            pending_mults.clear()
                            if cq > 0 and p == 0:
                                # previous chunk fully normalized now
                                for tt in range(4 * (cq - 1), 4 * cq):
                                    filler.append(
                                        ("out",
                                         lambda tt=tt: emit_outproj_tile(tt)))
                        # pipeline: next tile's scores go ahead of AV
                        if t + 1 < nts:
                            sts[t + 1] = emit_st(t + 1, p=p)
                        st, js = sts.pop(t)
                        pt = pt_pool.tile([P, 2, CH], BF16, tag="pt",
                                          name=f"pt{cq}_{p}_{t}")
                        nc.scalar.activation(pt[:, :, js:], st[:, :, js:], EXP)
                        for hh in range(2):
                            h = 2 * p + hh
                            nc.tensor.matmul(
                                ots[hh][:, js:],
                                lhsT=vaug_t[t][:, h, :],
                                rhs=pt[:, hh, js:],
                                start=(t == 0),
                                stop=(t == nts - 1),
                            )
                        drain(limit=2)
                    # ---- decoupled softmax normalization for (cq, p) ----
                    # bounce AV PSUM to SBUF right away (frees the ring)
                    otu = [small_pool.tile([D + 1, CH], F32, tag=f"otu{hh}",
                                           name=f"otu{cq}_{p}_{hh}")
                           for hh in range(2)]
                    for hh in range(2):
                        nc.vector.tensor_copy(otu[hh][:], ots[hh][:])
                    # denominator row to partition 0 via ACT (cross-partition
                    # copies are only safe on the scalar engine); read from
                    # the SBUF bounce so the PSUM ring frees on the DVE copy
                    den = small_pool.tile([1, 2, CH], F32, tag="dn",
                                          name=f"dn{cq}_{p}")
                    for hh in range(2):
                        nc.scalar.copy(den[:, hh, :], otu[hh][D : D + 1, :])
                    recd = small_pool.tile([1, 2, CH], F32, tag="rc",
                                           name=f"rc{cq}_{p}")
                    nc.vector.reciprocal_approx_fast(recd[:], den[:])
                    recb = small_pool.tile([D, 2, CH], F32, tag="rb",
                                           name=f"rb{cq}_{p}")
                    nc.gpsimd.partition_broadcast(recb[:], recd[:])

                    def mults(p=p, otu=otu, recb=recb, qsl=qsl):
                        with nc.allow_low_precision("bf16 store"):
                            for hh in range(2):
                                nc.vector.tensor_mul(
                                    ot_t[p][hh * D : (hh + 1) * D, qsl],
                                    otu[hh][0:D, :],
                                    recb[:, hh, :],
                                )
                    if no_defer:
                        mults()
                        if p == 1:
                            for tt in range(4 * cq, 4 * cq + 4):
                                filler.append(
                                    ("out", lambda tt=tt: emit_outproj_tile(tt)))
                    else:
                        pending_mults.append(mults)
                # next chunk's score matmuls read qt/kt of chunk cq+1:
                # force-drain any proj leftovers before emitting them
                drain(kinds=("proj",), limit=99)

            # ---- tail ----
            for fn in pending_mults:
                fn()
            pending_mults.clear()
            for tt in range(4 * (NCHUNK - 1), 4 * NCHUNK):
                filler.append(("out", lambda tt=tt: emit_outproj_tile(tt)))
            drain(limit=99)

    nc.finalize()
    return nc


_NC_CACHE = None


def get_nc():
    global _NC_CACHE
    if _NC_CACHE is None:
        _NC_CACHE = build_nc()
    return _NC_CACHE


def make_in_maps(x, Wq, Wk, Wv, Wo):
    scale = 1.0 / np.sqrt(np.float32(C))
    bf = ml_dtypes.bfloat16
    in_maps = []
    for core in range(8):
        b, hg = core // 4, core % 4
        hsl = slice(hg * NHPC, (hg + 1) * NHPC)
        xT = np.ascontiguousarray(x[b].T.astype(bf))
        wq = (Wq[hsl] * scale).transpose(1, 0, 2).reshape(C, DH)
        wk = Wk[hsl].transpose(1, 0, 2).reshape(C, DH)
        wv = Wv[hsl].transpose(1, 0, 2).reshape(C, DH)
        wqkv = np.ascontiguousarray(
            np.concatenate([wq, wk, wv], axis=1).astype(bf))
        wot = np.ascontiguousarray(Wo[:, hg * DH : (hg + 1) * DH].T.astype(bf))
        in_maps.append({
            "xT": xT,
            "Wqkv": wqkv,
            "WoT": wot,
        })
    return in_maps


def gather(results, bo):
    out = np.zeros((B, T, C), dtype=np.float32)
    for core in range(8):
        out[core // 4] += results[core]["Y"]
    out += bo.astype(np.float32)
    return out


def kernel(x, Wq, Wk, Wv, Wo, bo, **run_kwargs):
    x = np.asarray(x, dtype=np.float32)
    Wq = np.asarray(Wq, dtype=np.float32)
    Wk = np.asarray(Wk, dtype=np.float32)
    Wv = np.asarray(Wv, dtype=np.float32)
    Wo = np.asarray(Wo, dtype=np.float32)
    bo = np.asarray(bo, dtype=np.float32)
    nc = get_nc()
    in_maps = make_in_maps(x, Wq, Wk, Wv, Wo)
    res = run_bass_kernel_spmd(nc, in_maps, core_ids=list(range(8)), **run_kwargs)
    out = gather(res.results, bo)
    if run_kwargs:
        return out, res
    return out


# revision 62
# speedup vs baseline: 1.0301x; 1.0301x over previous
"""Trainium2 Bass kernel for 16-head causal MHA (B=2, T=2048, C=1024, H=16, D=64).

Sharding: 8 cores = 2 batch groups x 4 head groups (4 heads each).
All matmuls run in bf16 (inputs pre-cast on host; fp32 PSUM accumulate).

v3: single fused pipeline keeping the PE dense and the ACT-bound exp stream
overlapped:
  - projection matmul groups for chunk n+1 and out-projection tiles for
    chunk n-1 drain as PE "filler" inside chunk n's attention t-loops
  - causal mask applied on the PE (identity-matmul accumulate of a
    triangular bf16 constant into the score PSUM)
  - AV output PSUM freed immediately via an SBUF bounce copy; softmax
    normalization (reciprocal_approx_fast + gpsimd partition_broadcast +
    DVE multiply) runs decoupled, with the multiplies deferred into the
    next loop so no engine head-of-line blocks on the broadcast
  - PE warm-up matmuls flip the HAM clock gate to 8/8 during the initial
    DMA wait
Host sums the 4 head-group partials per batch and adds bo.
"""

import sys

sys.path.insert(0, "/opt/trn_rl_repo")

import numpy as np
import ml_dtypes

import concourse.bass as bass
from concourse import bacc
import concourse.mybir as mybir
from concourse.tile import TileContext
from concourse.bass_utils import run_bass_kernel_spmd
from concourse.masks import make_identity

F32 = mybir.dt.float32
BF16 = mybir.dt.bfloat16
F8 = mybir.dt.float8e4
DR = mybir.MatmulPerfMode.DoubleRow
EXP = mybir.ActivationFunctionType.Exp

B, T, C, H, D = 2, 2048, 1024, 16, 64
NHPC = 4          # heads per core
DH = NHPC * D     # 256 head dims per core
P = 128           # partitions
CH = 512          # token chunk (matmul moving dim)
NCHUNK = T // CH  # 4
NTT = T // P      # 16 token tiles
NCT = C // P      # 8 contraction tiles over C
NEG = -30000.0    # masked-score fill; exp() flushes to 0


import os

_DISABLE = set(os.environ.get("KERNEL_DISABLE", "").split(","))


def build_nc():
    no_warmup = "warmup" in _DISABLE
    no_defer = "defer" in _DISABLE
    no_interleave = "interleave" in _DISABLE
    no_pemask = "pemask" in _DISABLE
    nc = bacc.Bacc()
    xT_d = nc.declare_dram_parameter("xT", [C, T], BF16, isOutput=False)
    wqkv_d = nc.declare_dram_parameter("Wqkv", [C, 3 * DH], BF16, isOutput=False)
    wot_d = nc.declare_dram_parameter("WoT", [DH, C], BF16, isOutput=False)
    y_d = nc.declare_dram_parameter("Y", [T, C], F32, isOutput=True)

    # batched-DMA views: one descriptor chain covers a whole chunk / weight
    xTr = xT_d[:, :].rearrange("(c p) t -> p c t", p=P)      # [128, 8, 2048]
    wqkvr = wqkv_d[:, :].rearrange("(c p) d -> p c d", p=P)  # [128, 8, 768]
    wotr = wot_d[:, :].rearrange("(k p) e -> p k e", p=P)    # [128, 2, 1024]
    y = y_d[:, :]

    with TileContext(nc) as tc:
        with (
            tc.tile_pool(name="const", bufs=1) as const,
            tc.tile_pool(name="persist", bufs=1) as persist,
            tc.tile_pool(name="xt", bufs=4) as xt_pool,
            tc.tile_pool(name="pt", bufs=4) as pt_pool,
            tc.tile_pool(name="small", bufs=4) as small_pool,
            tc.tile_pool(name="ysb", bufs=4) as y_pool,
            tc.tile_pool(name="psproj", bufs=2, space="PSUM") as ps_proj,
            tc.tile_pool(name="psst", bufs=2, space="PSUM") as ps_st,
            tc.tile_pool(name="psot", bufs=2, space="PSUM") as ps_ot,
        ):
            # ---- persistent weight tiles + batched input DMAs (issued
            # before the const init so data streams during setup) ----
            wq_all = persist.tile([P, NCT, 3 * DH], BF16, name="wqall")
            wot_all = persist.tile([P, 2, C], BF16, name="wotall")
            xts = {}  # chunk -> [128, 8, 512] tile

            def emit_x_dma(n):
                xtile = xt_pool.tile([P, NCT, CH], BF16, tag="xt",
                                     name=f"xt{n}")
                nc.sync.dma_start(xtile[:],
                                  xTr[:, :, n * CH : (n + 1) * CH])
                xts[n] = xtile

            emit_x_dma(0)
            # split so Q/K projections can start before the V block lands
            nc.sync.dma_start(wq_all[:, :, 0 : 2 * DH],
                              wqkvr[:, :, 0 : 2 * DH])
            nc.sync.dma_start(wq_all[:, :, 2 * DH : 3 * DH],
                              wqkvr[:, :, 2 * DH : 3 * DH])
            nc.sync.dma_start(wot_all[:], wotr[:, :, :])
            emit_x_dma(1)

            # ---- constants ----
            id_f32 = const.tile([P, P], F32, name="idf")
            make_identity(nc, id_f32[:])
            id128 = const.tile([P, P], BF16, name="id128")
            # strictly-upper-triangular NEG (transposed causal mask):
            # maskT[c, i] = NEG if i > c else 0
            maskT_f32 = const.tile([P, P], F32, name="mtf")
            nc.gpsimd.memset(maskT_f32[:], 0.0)
            nc.gpsimd.affine_select(
                out=maskT_f32[:],
                in_=maskT_f32[:],
                compare_op=mybir.AluOpType.is_ge,
                fill=NEG,
                base=0,
                pattern=[[-1, P]],
                channel_multiplier=1,
            )
            maskT = const.tile([P, P], BF16, name="maskT")
            with nc.allow_low_precision("bf16 consts"):
                nc.vector.tensor_copy(id128[:], id_f32[:])
                nc.vector.tensor_copy(maskT[:], maskT_f32[:])
            ones_col = const.tile([1, D], F32, name="ones_col")
            nc.gpsimd.memset(ones_col[:], 1.0)
            if no_pemask:
                # DVE-side causal mask (baseline style):
                # mask128[r, (hh, j)] = 0 if r <= j else NEG
                mask128 = const.tile([P, 2, P], F32, name="mask128")
                nc.gpsimd.memset(mask128[:], 0.0)
                nc.gpsimd.affine_select(
                    out=mask128[:],
                    in_=mask128[:],
                    compare_op=mybir.AluOpType.is_ge,
                    fill=NEG,
                    base=0,
                    pattern=[[0, 2], [1, P]],
                    channel_multiplier=-1,
                )

            # ---- persistent tensors ----
            # Q^T/K^T [dims, tokens]; pair p holds heads (2p, 2p+1)
            qt_t = [persist.tile([P, T], BF16, name=f"qt{p}") for p in range(2)]
            kt_t = [persist.tile([P, T], BF16, name=f"kt{p}") for p in range(2)]
            # V augmented with a ones column per head: [tokens, 4, 65]
            vaug_t = [persist.tile([P, NHPC, D + 1], BF16, name=f"vaug{t}")
                      for t in range(NTT)]
            for t in range(NTT):
                nc.gpsimd.memset(vaug_t[t][:, :, D : D + 1], 1.0)
            # normalized O^T [dims, tokens]
            ot_t = [persist.tile([P, T], BF16, name=f"ot{p}") for p in range(2)]

            # ---- PE warm-up (runs during the initial DMA wait) ----
            if not no_warmup:
                warm = ps_proj.tile([P, CH], F32, tag="pp", name="warm")
                for _ in range(72):
                    nc.tensor.matmul(warm[:, 0:P], lhsT=id128[:], rhs=maskT[:],
                                     start=True, stop=True)

            # ---- stage-1 projection groups (PE filler units) ----
            def emit_proj_qk(n, m):
                csl = slice(n * CH, (n + 1) * CH)
                ps = ps_proj.tile([P, CH], F32, tag="pp", name=f"ps{n}_{m}")
                for c in range(NCT):
                    nc.tensor.matmul(
                        ps[:],
                        lhsT=wq_all[:, c, m * P : (m + 1) * P],
                        rhs=xts[n][:, c, :],
                        start=(c == 0),
                        stop=(c == NCT - 1),
                    )
                with nc.allow_low_precision("bf16 store"):
                    if m < 2:
                        nc.vector.tensor_copy(qt_t[m][:, csl], ps[:])
                    else:
                        nc.vector.tensor_copy(kt_t[m - 2][:, csl], ps[:])

            def emit_proj_v(n, j):
                vp = ps_proj.tile([P, CH], F32, tag="pp", name=f"vp{n}_{j}")
                for c in range(NCT):
                    nc.tensor.matmul(
                        vp[:, 0:DH],
                        lhsT=xts[n][:, c, j * P : (j + 1) * P],
                        rhs=wq_all[:, c, 2 * DH : 3 * DH],
                        start=(c == 0),
                        stop=(c == NCT - 1),
                    )
                va = vaug_t[4 * n + j]
                with nc.allow_low_precision("bf16 store"):
                    nc.vector.tensor_copy(
                        va[:, :, 0:D],
                        vp[:, 0:DH].rearrange("p (h d) -> p h d", h=NHPC))

            def proj_chunk_units(n):
                units = [(lambda m=m, n=n: emit_proj_qk(n, m)) for m in range(4)]
                units += [(lambda j=j, n=n: emit_proj_v(n, j)) for j in range(4)]
                return units

            # chunk 0 runs before the attention loops start
            for u in proj_chunk_units(0):
                u()

            # ---- out-projection tile (PE filler unit) ----
            ysb_tiles = {}

            def emit_outproj_half(tt, nn):
                tsl = slice(tt * P, (tt + 1) * P)
                if nn == 0:
                    ysb_tiles[tt] = y_pool.tile([P, 2, CH], F32, tag="ysb",
                                                name=f"ysb{tt}")
                ysb = ysb_tiles[tt]
                nsl = slice(nn * CH, (nn + 1) * CH)
                yp = ps_proj.tile([P, CH], F32, tag="pp", name=f"y{tt}_{nn}")
                for k in range(2):
                    nc.tensor.matmul(
                        yp[:],
                        lhsT=ot_t[k][:, tsl],
                        rhs=wot_all[:, k, nsl],
                        start=(k == 0),
                        stop=(k == 1),
                    )
                nc.vector.tensor_copy(ysb[:, nn, :], yp[:])
                if nn == 1:
                    nc.sync.dma_start(y[tsl, :], ysb[:])
                    del ysb_tiles[tt]

            def emit_outproj_tile(tt):
                emit_outproj_half(tt, 0)
                emit_outproj_half(tt, 1)

            # ---- fused attention with filler drain ----
            filler = []          # list of (kind, emit_fn)
            pending_mults = []   # deferred normalization multiplies

            def drain(kinds=("proj", "out"), limit=1):
                done = 0
                i = 0
                while i < len(filler) and done < limit:
                    kind, fn = filler[i]
                    if kind in kinds:
                        filler.pop(i)
                        fn()
                        done += 1
                    else:
                        i += 1

            for cq in range(NCHUNK):
                if cq + 1 <= NCHUNK - 1:
                    units = proj_chunk_units(cq + 1)
                    if no_interleave:
                        for u in units:
                            u()
                    else:
                        filler.extend(("proj", u) for u in units)
                if cq + 2 <= NCHUNK - 1:
                    emit_x_dma(cq + 2)
                qsl = slice(cq * CH, (cq + 1) * CH)
                nts = 4 * cq + 4

                def emit_st(t, cq=cq, p=None):
                    st = ps_st.tile([P, 2, CH], F32, tag="st",
                                    name=f"st{cq}_{p}_{t}")
                    tsl = slice(t * P, (t + 1) * P)
                    js = max(0, (t - 4 * cq) * P)
                    qs = slice(cq * CH + js, (cq + 1) * CH)
                    diag = t >= 4 * cq
                    for hh in range(2):
                        nc.tensor.matmul(
                            st[:, hh, js:],
                            lhsT=kt_t[p][hh * D : (hh + 1) * D, tsl],
                            rhs=qt_t[p][hh * D : (hh + 1) * D, qs],
                            start=True,
                            stop=True if no_pemask else not diag,
                        )
                    if diag:
                        if no_pemask:
                            nc.vector.tensor_add(
                                st[:, :, js : js + P],
                                st[:, :, js : js + P],
                                mask128[:],
                            )
                        else:
                            # causal mask on the PE: st[:,hh,js:js+P] += maskT.T
                            for hh in range(2):
                                nc.tensor.matmul(
                                    st[:, hh, js : js + P],
                                    lhsT=maskT[:],
                                    rhs=id128[:],
                                    start=False,
                                    stop=True,
                                )
                    return st, js

                for p in range(2):
                    ots = [
                        ps_ot.tile([D + 1, CH], F32, tag="ot",
                                   name=f"ot{cq}_{p}_{hh}")
                        for hh in range(2)
                    ]
                    sts = {0: emit_st(0, p=p)}
                    for t in range(nts):
                        if t == 2 and pending_mults:
                            for fn in pending_mults:
                                fn()
                            pending_mults.clear()
                            if cq > 0 and p == 0:
                                # previous chunk fully normalized now
                                for tt in range(4 * (cq - 1), 4 * cq):
                                    filler.append(
                                        ("out",
                                         lambda tt=tt: emit_outproj_tile(tt)))
                        # pipeline: next tile's scores go ahead of AV
                        if t + 1 < nts:
                            sts[t + 1] = emit_st(t + 1, p=p)
                        st, js = sts.pop(t)
                        pt = pt_pool.tile([P, 2, CH], BF16, tag="pt",
                                          name=f"pt{cq}_{p}_{t}")
                        nc.scalar.activation(pt[:, :, js:], st[:, :, js:], EXP)
                        for hh in range(2):
                            h = 2 * p + hh
                            nc.tensor.matmul(
                                ots[hh][:, js:],
                                lhsT=vaug_t[t][:, h, :],
                                rhs=pt[:, hh, js:],
                                start=(t == 0),
                                stop=(t == nts - 1),
                            )
                        drain(limit=2 if cq == 0 else 1)
                    # ---- decoupled softmax normalization for (cq, p) ----
                    # bounce AV PSUM to SBUF right away (frees the ring)
                    otu = [small_pool.tile([D + 1, CH], F32, tag=f"otu{hh}",
                                           name=f"otu{cq}_{p}_{hh}")
                           for hh in range(2)]
                    for hh in range(2):
                        nc.vector.tensor_copy(otu[hh][:], ots[hh][:])
                    # denominator row to partition 0 via ACT (cross-partition
                    # copies are only safe on the scalar engine); read from
                    # the SBUF bounce so the PSUM ring frees on the DVE copy
                    den = small_pool.tile([1, 2, CH], F32, tag="dn",
                                          name=f"dn{cq}_{p}")
                    for hh in range(2):
                        nc.scalar.copy(den[:, hh, :], otu[hh][D : D + 1, :])
                    recd = small_pool.tile([1, 2, CH], F32, tag="rc",
                                           name=f"rc{cq}_{p}")
                    nc.vector.reciprocal_approx_fast(recd[:], den[:])
                    recb = small_pool.tile([D, 2, CH], F32, tag="rb",
                                           name=f"rb{cq}_{p}")
                    nc.gpsimd.partition_broadcast(recb[:], recd[:])

                    def mults(p=p, otu=otu, recb=recb, qsl=qsl):
                        with nc.allow_low_precision("bf16 store"):
                            for hh in range(2):
                                nc.vector.tensor_mul(
                                    ot_t[p][hh * D : (hh + 1) * D, qsl],
                                    otu[hh][0:D, :],
                                    recb[:, hh, :],
                                )
                    if no_defer:
                        mults()
                        if p == 1:
                            for tt in range(4 * cq, 4 * cq + 4):
                                filler.append(
                                    ("out", lambda tt=tt: emit_outproj_tile(tt)))
                    else:
                        pending_mults.append(mults)
                # next chunk's score matmuls read qt/kt of chunk cq+1:
                # force-drain any proj leftovers before emitting them
                drain(kinds=("proj",), limit=99)

            # ---- tail ----
            for fn in pending_mults:
                fn()
            pending_mults.clear()
            for tt in range(4 * (NCHUNK - 1), 4 * NCHUNK):
                filler.append(("out", lambda tt=tt: emit_outproj_tile(tt)))
            drain(limit=99)

    nc.finalize()
    return nc


_NC_CACHE = None


def get_nc():
    global _NC_CACHE
    if _NC_CACHE is None:
        _NC_CACHE = build_nc()
    return _NC_CACHE


def make_in_maps(x, Wq, Wk, Wv, Wo):
    scale = 1.0 / np.sqrt(np.float32(C))
    bf = ml_dtypes.bfloat16
    in_maps = []
    for core in range(8):
        b, hg = core // 4, core % 4
        hsl = slice(hg * NHPC, (hg + 1) * NHPC)
        xT = np.ascontiguousarray(x[b].T.astype(bf))
        wq = (Wq[hsl] * scale).transpose(1, 0, 2).reshape(C, DH)
        wk = Wk[hsl].transpose(1, 0, 2).reshape(C, DH)
        wv = Wv[hsl].transpose(1, 0, 2).reshape(C, DH)
        wqkv = np.ascontiguousarray(
            np.concatenate([wq, wk, wv], axis=1).astype(bf))
        wot = np.ascontiguousarray(Wo[:, hg * DH : (hg + 1) * DH].T.astype(bf))
        in_maps.append({
            "xT": xT,
            "Wqkv": wqkv,
            "WoT": wot,
        })
    return in_maps


def gather(results, bo):
    out = np.zeros((B, T, C), dtype=np.float32)
    for core in range(8):
        out[core // 4] += results[core]["Y"]
    out += bo.astype(np.float32)
    return out


def kernel(x, Wq, Wk, Wv, Wo, bo, **run_kwargs):
    x = np.asarray(x, dtype=np.float32)
    Wq = np.asarray(Wq, dtype=np.float32)
    Wk = np.asarray(Wk, dtype=np.float32)
    Wv = np.asarray(Wv, dtype=np.float32)
    Wo = np.asarray(Wo, dtype=np.float32)
    bo = np.asarray(bo, dtype=np.float32)
    nc = get_nc()
    in_maps = make_in_maps(x, Wq, Wk, Wv, Wo)
    res = run_bass_kernel_spmd(nc, in_maps, core_ids=list(range(8)), **run_kwargs)
    out = gather(res.results, bo)
    if run_kwargs:
        return out, res
    return out
